# revision 29
# baseline (speedup 1.0000x reference)
"""DTW distance kernel for Trainium2 (8 NeuronCores, SPMD data-parallel over batch).

Per core: NB=16 batch elements. Host precomputes (inside kernel(), cheap numpy):
  xm2 = -2x as bf16 [F, nb, T], yb = y as bf16 [F, nb, T],
  y2[j] = sum_f y^2 as bf16 [1, nb*T], x2[i] = sum_f x^2 as f32 [CM, nb*NC]
  (x2 laid out as the ACT sqrt's per-partition bias columns).
Phase 1 (cost matrix): cost[b][i,j] = ||x[b,:,i] - y[b,:,j]||_2.
  d2 = xm2_chunk^T yb + ones^T y2 via 2 accumulated bf16 PE matmuls per
  128-row chunk; cost = ACT sqrt(psum + x2col) -> bf16 staging -> DRAM
  scratch (skew absorbed into a linear stride), then streamed back into the
  c-block of a [Z-block | c-block] SBUF ring in big per-window DMAs.
Phase 2 (DP): dtw wavefront. 8 column-strips x 16 batches = 128 partitions
  (partition p = s*16 + b). Strip s lags strip s-1 by L steps. Per step t
  (strip s handles row i = t - L*s) ONE fused DVE scan over 2W positions:
    pos 2j  : state = min(prev_row[j-1], state) + 0      (zero from Z-block)
    pos 2j+1: state = min(prev_row[j],   state) + c_j    (cost from c-block)
  i.e. S_j = c_j + min(chain, R[j-1], R[j]) -- the 3-way DTW min with the
  neighbour min folded into the scan via a doubled (stride-2) data0 AP.
  Row layout (C = 2W+2 cols): col0 = bnd (cross-strip boundary), col 2+2j =
  S_j, col 1+2j = dump (even-position scan writes). The scan's init reads
  col0 of the current slot; data0's pos0 reads col0 of the previous slot --
  left and diagonal boundary values both enter via col0.
  Cross-strip boundaries: one small SBUF->SBUF DMA per kb steps copies strip
  s-1's col-2W (last S) values into strip s's col0, shifted +nb partitions
  and +L slots (lead = L-kb+1 steps hides DMA latency). Strip-0 col0 stays
  BIG; t=0 uses a zero init via zcol.
  ring and rows are hand-placed at 256B-aligned SBUF offsets: a misaligned
  (mod64=32) scan src1 base costs ~100ns per scan (measured).
  Inactive strip lanes stay huge (>=1e30): ring head c-blocks are BIG, rows
  init to BIG, so garbage never contaminates valid lanes.
"""
import sys
import numpy as np

sys.path.insert(0, "/opt/trn_rl_repo")

import concourse.bass as bass  # noqa: E402
import concourse.bacc as bacc  # noqa: E402
import concourse.mybir as mybir  # noqa: E402
import concourse.tile as tile  # noqa: E402

NCORES = 8
B_FULL, F_FULL, T_FULL = 128, 128, 512
BIG = 1.0e30


def ap_at(h, offset, dims):
    """AP on tensor handle h at element offset with explicit [stride,count] dims."""
    return bass.AP(tensor=h, offset=offset, ap=[list(d) for d in dims])


def build_dtw(nb, F, T, S, W, L, nslot=16, wt=64, kb=2, nring=6):
    """Per-core SPMD Bass graph. Partition p = s*nb + b."""
    assert S * W == T and S * nb <= 128
    assert L % kb == 0 and nslot % kb == 0 and nslot > L + kb + 2
    P = S * nb
    NC = (T + 127) // 128
    CM = T // NC
    assert CM * NC == T
    TS = L * (S - 1) + T                  # DP steps
    TR = nring * wt                       # cost ring rows
    C = 2 * W + 2                         # striped row width
    f32, bf16 = mybir.dt.float32, mybir.dt.bfloat16
    mn, ad = mybir.AluOpType.min, mybir.AluOpType.add
    AF = mybir.ActivationFunctionType

    nc = bacc.Bacc(None, target_bir_lowering=False, debug=False)
    xm2 = nc.declare_dram_parameter("xm2", [F, nb, T], bf16, isOutput=False)
    yb = nc.declare_dram_parameter("yb", [F, nb, T], bf16, isOutput=False)
    y2 = nc.declare_dram_parameter("y2", [1, nb * T], bf16, isOutput=False)
    x2 = nc.declare_dram_parameter("x2", [CM, nb * NC], f32, isOutput=False)
    out = nc.declare_dram_parameter("out", [nb, 1], f32, isOutput=True)
    scratch = nc.dram_tensor("scratch", [P * TS * W], bf16)

    def scr_ap(offset, dims):
        return bass.AP(tensor=scratch, offset=offset, ap=[list(d) for d in dims])

    # hand-placed, 256B-aligned ring + rows near the top of SBUF
    # (below nc.sbuf_top, which excludes the TRN2 evtaccel reservation)
    ring_bytes = (TR + 1) * W * 2         # bf16, shared Z block + c rows
    rows_bytes = ((nslot * C * 4 + 255) // 256) * 256
    ring_off = ((nc.sbuf_top - ring_bytes) // 256) * 256
    rows_off = ring_off - rows_bytes
    nc.sbuf_top = rows_off                # fence the arena from the allocator
    ring_h = nc.alloc_sbuf_tensor_at("ringm", [P, (TR + 1) * W], bf16,
                                     offset=ring_off)
    rows_h = nc.alloc_sbuf_tensor_at("rowsm", [P, nslot, C], f32,
                                     offset=rows_off)
    rpst = (TR + 1) * W
    # ring view: [P, 1+TR, W] -- row 0 is the shared Z block, c-row r at 1+r
    ring = ap_at(ring_h, 0, [[rpst, P], [W, TR + 1], [1, W]])
    rows = ap_at(rows_h, 0, [[nslot * C, P], [C, nslot], [1, C]])
    pst = nslot * C

    with tile.TileContext(nc) as tc:
        with (
            tc.tile_pool(name="persist", bufs=1) as pp,
            tc.tile_pool(name="stg", bufs=6) as stgp,
            tc.tile_pool(name="ps_d2", bufs=4, space="PSUM") as psd,
            tc.tile_pool(name="ps_h", bufs=4, space="PSUM") as psh,
        ):
            # ---- constants / init ----
            ones_row = pp.tile([1, T], bf16, tag="ones_row")
            nc.vector.memset(ones_row[:], 1.0)
            zcol = pp.tile([P, 1], f32, tag="zcol")
            nc.vector.memset(zcol[:], BIG)
            nc.vector.memset(zcol[0:nb, 0:1], 0.0)
            # ring: shared Z block zeros, head c-rows BIG
            nc.vector.memset(ring[:, 0:1, :], 0.0)
            nc.vector.memset(ring[:, 1:1 + L * (S - 1), :], BIG)
            nc.vector.memset(rows[:, :, :], BIG)

            # ---- inputs: small consts first, then grouped batch loads ----
            txm2 = pp.tile([F, nb, T], bf16, tag="txm2")
            tyb = pp.tile([F, nb, T], bf16, tag="tyb")
            ty2 = pp.tile([1, nb * T], bf16, tag="ty2")
            tx2 = pp.tile([CM, nb * NC], f32, tag="tx2")
            LG = 4
            nc.scalar.dma_start(txm2[:, 0:2, :], xm2[:, 0:2, :])
            nc.scalar.dma_start(tyb[:, 0:2, :], yb[:, 0:2, :])
            nc.scalar.dma_start(ty2[:], y2[:, :])
            nc.scalar.dma_start(tx2[:], x2[:, :])
            nc.scalar.dma_start(txm2[:, 2:LG, :], xm2[:, 2:LG, :])
            nc.scalar.dma_start(tyb[:, 2:LG, :], yb[:, 2:LG, :])
            for g0 in range(LG, nb, LG):
                nc.scalar.dma_start(txm2[:, g0:g0 + LG, :],
                                    xm2[:, g0:g0 + LG, :])
                nc.scalar.dma_start(tyb[:, g0:g0 + LG, :],
                                    yb[:, g0:g0 + LG, :])

            # PE p-state warmup: keep PE busy while loads land so the
            # produce matmuls run at full clock (PE idles at a lower p-state)
            for _ in range(8):
                wps = psh.tile([CM, 2 * W], f32, tag="warm")
                nc.tensor.matmul(wps[:], ones_row[0:1, 0:CM],
                                 ones_row[0:1, 0:2 * W], start=True, stop=True)

            def produce_cols(c, b, s0, s1, pool, dmaeng=None):
                cw = (s1 - s0) * W
                ps = pool.tile([CM, cw], f32, tag=f"ps{cw}")
                nc.tensor.matmul(ps[:], txm2[:, b, c * CM:(c + 1) * CM],
                                 tyb[:, b, s0 * W:s1 * W],
                                 start=True, stop=False)
                nc.tensor.matmul(ps[:], ones_row[0:1, c * CM:(c + 1) * CM],
                                 ty2[0:1, b * T + s0 * W:b * T + s1 * W],
                                 start=False, stop=True)
                stg = stgp.tile([CM, cw], bf16, tag=f"stg{cw}")
                nc.scalar.activation(
                    stg[:], ps[:], AF.Sqrt,
                    bias=tx2[:, b * NC + c:b * NC + c + 1], scale=1.0)
                # write skewed: addr(b; i, s, f) =
                #   (s*nb+b)*TS*W + (L*s + c*CM + i)*W + f
                (dmaeng or nc.scalar).dma_start(
                    scr_ap(b * TS * W + c * CM * W
                           + s0 * (nb * TS + L) * W,
                           [[W, CM], [(nb * TS + L) * W, s1 - s0], [1, W]]),
                    stg[:])


            # ---- DP loop: one fused scan per step + boundary DMA per kb ----
            def dp_step(t):
                slot, pslot = t % nslot, (t - 1) % nslot
                data0 = ap_at(rows_h, pslot * C, [[pst, P], [2, W], [2, 2]])
                data1 = ap_at(ring_h, 0,
                              [[rpst, P], [1, W], [(1 + t % TR) * W, 2]])
                outap = ap_at(rows_h, slot * C + 1, [[pst, P], [1, 2 * W]])
                init = (zcol[:, 0:1] if t == 0 else rows[:, slot, 0:1])
                eng = nc.vector
                eng.add_instruction(
                    mybir.InstTensorScalarPtr(
                        name=nc.get_next_instruction_name(),
                        is_tensor_tensor_scan=True,
                        is_scalar_tensor_tensor=True,
                        op0=mn, op1=ad,
                        ins=[eng.lower_ap(data0),
                             eng.lower_ap_or_imm(init),
                             eng.lower_ap(data1)],
                        outs=[eng.lower_ap(outap)],
                    ))
                # boundary DMA for scans [u, u+kb): strip s-1 col-2W -> strip
                # s col0, +nb partitions, +L slots. Sources complete at this t.
                u = t + L - (kb - 1)
                if u >= L and u % kb == 0 and u < TS:
                    ssl = (u - L) % nslot
                    dsl = u % nslot
                    with nc.allow_non_contiguous_dma(
                            reason="tiny boundary column copy, kb elems/part"):
                        nc.sync.dma_start(
                            rows[nb:P, dsl:dsl + kb, 0:1],
                            rows[0:P - nb, ssl:ssl + kb, 2 * W:2 * W + 1])

            # sub-window reads (SW steps each), spread across the DP so the
            # DMA engines never burst long enough to delay a boundary packet
            SW = 16
            RA = TR - wt          # read-ahead in steps (ring wrap safe)
            n_sw = (TS + SW - 1) // SW
            prod_c = 1

            def sub_read(v):
                t0, t1 = v * SW, min((v + 1) * SW, TS)
                r0 = t0 % TR
                full = [s for s in range(S)
                        if L * s <= t0 and L * s + T >= t1]
                if full:
                    s_a, s_b = min(full), max(full)
                    nc.scalar.dma_start(
                        ring[s_a * nb:(s_b + 1) * nb, 1 + r0:1 + r0 + (t1 - t0), :],
                        scr_ap(s_a * nb * TS * W + t0 * W,
                               [[TS * W, (s_b - s_a + 1) * nb],
                                [1, (t1 - t0) * W]]))
                for s in range(S):
                    if s in full:
                        continue
                    v0, v1 = max(t0, L * s), min(t1, L * s + T)
                    if v0 >= v1:
                        continue
                    nc.scalar.dma_start(
                        ring[s * nb:(s + 1) * nb,
                             1 + r0 + (v0 - t0):1 + r0 + (v1 - t0), :],
                        scr_ap(s * nb * TS * W + v0 * W,
                               [[TS * W, nb], [1, (v1 - v0) * W]]))

            # chunk 0 full width gates the first scans; chunks 1-3 paced
            # inside the DP loop; sub-reads follow their producer writes
            for b in range(nb):
                produce_cols(0, b, 0, S, psd, dmaeng=nc.sync)

            def sw_ready(v):
                t1p = min((v + 1) * SW, TS)
                ch = min(NC - 1, (t1p - 1) // CM)
                return ch == 0 or prods_done >= ch * nb

            prods = [(c, b) for c in range(1, NC) for b in range(nb)]
            prods_done = 0
            sw_read = 0
            while sw_read * SW < 2 * wt and sw_ready(sw_read):
                sub_read(sw_read)
                sw_read += 1
            for t in range(TS):
                dp_step(t)
                if t % 5 == 2 and prods:
                    produce_cols(*prods.pop(0), 0, S, psd)
                    prods_done += 1
                if t % 2 == 1:
                    if (sw_read < n_sw and sw_read * SW < t + RA
                            and sw_ready(sw_read)):
                        sub_read(sw_read)
                        sw_read += 1

            # ---- extract answers: strip S-1, row T-1, col 2W ----
            nc.sync.dma_start(
                out[:], rows[(S - 1) * nb:P, (TS - 1) % nslot,
                             2 * W:2 * W + 1])

    nc.compile()
    return nc


_cache = {}

NB = B_FULL // NCORES
S_, W_, L_ = 8, 64, 8
NC_ = (T_FULL + 127) // 128
CM_ = T_FULL // NC_


def _get_nc():
    key = "full"
    if key not in _cache:
        _cache[key] = build_dtw(
            nb=NB, F=F_FULL, T=T_FULL, S=S_, W=W_, L=L_)
    return _cache[key]


def make_in_maps(x, y):
    """Host prep: shard over cores, transpose to [F, nb, T], cast bf16,
    precompute x2/y2 sums of squares in the kernel's layouts."""
    from ml_dtypes import bfloat16
    x = np.ascontiguousarray(x, dtype=np.float32)
    y = np.ascontiguousarray(y, dtype=np.float32)
    in_maps = []
    for c in range(NCORES):
        xs = x[c * NB:(c + 1) * NB]                      # [nb, F, T]
        ys = y[c * NB:(c + 1) * NB]
        xm2 = np.ascontiguousarray(
            (-2.0 * xs).transpose(1, 0, 2)).astype(bfloat16)
        yb = np.ascontiguousarray(ys.transpose(1, 0, 2)).astype(bfloat16)
        y2 = np.einsum("bft,bft->bt", ys, ys).reshape(1, -1).astype(bfloat16)
        x2b = np.einsum("bft,bft->bt", xs, xs)           # [nb, T]
        # x2 bias layout: [CM, nb*NC], column b*NC+c = x2[b, c*CM:(c+1)*CM]
        x2l = np.ascontiguousarray(
            x2b.reshape(NB, NC_, CM_).transpose(2, 0, 1)
        ).reshape(CM_, NB * NC_).astype(np.float32)
        in_maps.append({"xm2": xm2, "yb": yb, "y2": y2,
                        "x2": np.ascontiguousarray(x2l)})
    return in_maps


def kernel(x, y):
    from concourse.bass_utils import run_bass_kernel_spmd

    nc = _get_nc()
    in_maps = make_in_maps(x, y)
    res = run_bass_kernel_spmd(nc, in_maps, list(range(NCORES)))
    outs = [res.results[c]["out"].reshape(NB) for c in range(NCORES)]
    return np.concatenate(outs).astype(np.float32)


# revision 30
# speedup vs baseline: 1.0143x; 1.0143x over previous
"""DTW distance kernel for Trainium2 (8 NeuronCores, SPMD data-parallel over batch).

Per core: NB=16 batch elements. Host precomputes (inside kernel(), cheap numpy):
  xm2 = -2x as bf16 [F, nb, T], yb = y as bf16 [F, nb, T],
  y2[j] = sum_f y^2 as bf16 [1, nb*T], x2[i] = sum_f x^2 as f32 [CM, nb*NC]
  (x2 laid out as the ACT sqrt's per-partition bias columns).
Phase 1 (cost matrix): cost[b][i,j] = ||x[b,:,i] - y[b,:,j]||_2.
  d2 = xm2_chunk^T yb + ones^T y2 via 2 accumulated bf16 PE matmuls per
  128-row chunk; cost = ACT sqrt(psum + x2col) -> bf16 staging -> DRAM
  scratch (skew absorbed into a linear stride), then streamed back into the
  c-block of a [Z-block | c-block] SBUF ring in big per-window DMAs.
Phase 2 (DP): dtw wavefront. 8 column-strips x 16 batches = 128 partitions
  (partition p = s*16 + b). Strip s lags strip s-1 by L steps. Per step t
  (strip s handles row i = t - L*s) ONE fused DVE scan over 2W positions:
    pos 2j  : state = min(prev_row[j-1], state) + 0      (zero from Z-block)
    pos 2j+1: state = min(prev_row[j],   state) + c_j    (cost from c-block)
  i.e. S_j = c_j + min(chain, R[j-1], R[j]) -- the 3-way DTW min with the
  neighbour min folded into the scan via a doubled (stride-2) data0 AP.
  Row layout (C = 2W+2 cols): col0 = bnd (cross-strip boundary), col 2+2j =
  S_j, col 1+2j = dump (even-position scan writes). The scan's init reads
  col0 of the current slot; data0's pos0 reads col0 of the previous slot --
  left and diagonal boundary values both enter via col0.
  Cross-strip boundaries: one small SBUF->SBUF DMA per kb steps copies strip
  s-1's col-2W (last S) values into strip s's col0, shifted +nb partitions
  and +L slots (lead = L-kb+1 steps hides DMA latency). Strip-0 col0 stays
  BIG; t=0 uses a zero init via zcol.
  ring and rows are hand-placed at 256B-aligned SBUF offsets: a misaligned
  (mod64=32) scan src1 base costs ~100ns per scan (measured).
  Inactive strip lanes stay huge (>=1e30): ring head c-blocks are BIG, rows
  init to BIG, so garbage never contaminates valid lanes.
"""
import sys
import numpy as np

sys.path.insert(0, "/opt/trn_rl_repo")

import concourse.bass as bass  # noqa: E402
import concourse.bacc as bacc  # noqa: E402
import concourse.mybir as mybir  # noqa: E402
import concourse.tile as tile  # noqa: E402

NCORES = 8
B_FULL, F_FULL, T_FULL = 128, 128, 512
BIG = 1.0e30


def ap_at(h, offset, dims):
    """AP on tensor handle h at element offset with explicit [stride,count] dims."""
    return bass.AP(tensor=h, offset=offset, ap=[list(d) for d in dims])


def build_dtw(nb, F, T, S, W, L, nslot=16, wt=64, kb=2, nring=6):
    """Per-core SPMD Bass graph. Partition p = s*nb + b."""
    assert S * W == T and S * nb <= 128
    assert L % kb == 0 and nslot % kb == 0 and nslot > L + kb + 2
    P = S * nb
    NC = (T + 127) // 128
    CM = T // NC
    assert CM * NC == T
    TS = L * (S - 1) + T                  # DP steps
    TR = nring * wt                       # cost ring rows
    C = 2 * W + 2                         # striped row width
    f32, bf16 = mybir.dt.float32, mybir.dt.bfloat16
    mn, ad = mybir.AluOpType.min, mybir.AluOpType.add
    AF = mybir.ActivationFunctionType

    nc = bacc.Bacc(None, target_bir_lowering=False, debug=False)
    xm2 = nc.declare_dram_parameter("xm2", [F, nb, T], bf16, isOutput=False)
    yb = nc.declare_dram_parameter("yb", [F, nb, T], bf16, isOutput=False)
    y2 = nc.declare_dram_parameter("y2", [1, nb * T], bf16, isOutput=False)
    x2 = nc.declare_dram_parameter("x2", [CM, nb * NC], f32, isOutput=False)
    out = nc.declare_dram_parameter("out", [nb, 1], f32, isOutput=True)
    scratch = nc.dram_tensor("scratch", [P * TS * W], bf16)

    def scr_ap(offset, dims):
        return bass.AP(tensor=scratch, offset=offset, ap=[list(d) for d in dims])

    # hand-placed, 256B-aligned ring + rows near the top of SBUF
    # (below nc.sbuf_top, which excludes the TRN2 evtaccel reservation)
    ring_bytes = (TR + 1) * W * 2         # bf16, shared Z block + c rows
    rows_bytes = ((nslot * C * 4 + 255) // 256) * 256
    ring_off = ((nc.sbuf_top - ring_bytes) // 256) * 256
    rows_off = ring_off - rows_bytes
    nc.sbuf_top = rows_off                # fence the arena from the allocator
    ring_h = nc.alloc_sbuf_tensor_at("ringm", [P, (TR + 1) * W], bf16,
                                     offset=ring_off)
    rows_h = nc.alloc_sbuf_tensor_at("rowsm", [P, nslot, C], f32,
                                     offset=rows_off)
    rpst = (TR + 1) * W
    # ring view: [P, 1+TR, W] -- row 0 is the shared Z block, c-row r at 1+r
    ring = ap_at(ring_h, 0, [[rpst, P], [W, TR + 1], [1, W]])
    rows = ap_at(rows_h, 0, [[nslot * C, P], [C, nslot], [1, C]])
    pst = nslot * C

    with tile.TileContext(nc) as tc:
        with (
            tc.tile_pool(name="persist", bufs=1) as pp,
            tc.tile_pool(name="stg", bufs=6) as stgp,
            tc.tile_pool(name="ps_d2", bufs=4, space="PSUM") as psd,
            tc.tile_pool(name="ps_h", bufs=4, space="PSUM") as psh,
        ):
            # ---- constants / init ----
            ones_row = pp.tile([1, T], bf16, tag="ones_row")
            nc.vector.memset(ones_row[:], 1.0)
            zcol = pp.tile([P, 1], f32, tag="zcol")
            nc.vector.memset(zcol[:], BIG)
            nc.vector.memset(zcol[0:nb, 0:1], 0.0)
            # ring: shared Z block zeros, head c-rows BIG
            nc.vector.memset(ring[:, 0:1, :], 0.0)
            nc.vector.memset(ring[:, 1:1 + L * (S - 1), :], BIG)
            nc.vector.memset(rows[:, :, :], BIG)

            # ---- inputs: small consts first, then grouped batch loads ----
            txm2 = pp.tile([F, nb, T], bf16, tag="txm2")
            tyb = pp.tile([F, nb, T], bf16, tag="tyb")
            ty2 = pp.tile([1, nb * T], bf16, tag="ty2")
            tx2 = pp.tile([CM, nb * NC], f32, tag="tx2")
            LG = 4
            nc.scalar.dma_start(ty2[:], y2[:, :])
            nc.scalar.dma_start(tx2[:], x2[:, :])
            nc.scalar.dma_start(txm2[:, 0:LG, :], xm2[:, 0:LG, :])
            nc.scalar.dma_start(tyb[:, 0:LG, :], yb[:, 0:LG, :])
            for g0 in range(LG, nb, LG):
                nc.scalar.dma_start(txm2[:, g0:g0 + LG, :],
                                    xm2[:, g0:g0 + LG, :])
                nc.scalar.dma_start(tyb[:, g0:g0 + LG, :],
                                    yb[:, g0:g0 + LG, :])

            # PE p-state warmup: keep PE busy while loads land so the
            # produce matmuls run at full clock (PE idles at a lower p-state)
            for _ in range(8):
                wps = psh.tile([CM, 2 * W], f32, tag="warm")
                nc.tensor.matmul(wps[:], ones_row[0:1, 0:CM],
                                 ones_row[0:1, 0:2 * W], start=True, stop=True)

            def produce_cols(c, b, s0, s1, pool, dmaeng=None):
                cw = (s1 - s0) * W
                ps = pool.tile([CM, cw], f32, tag=f"ps{cw}")
                nc.tensor.matmul(ps[:], txm2[:, b, c * CM:(c + 1) * CM],
                                 tyb[:, b, s0 * W:s1 * W],
                                 start=True, stop=False)
                nc.tensor.matmul(ps[:], ones_row[0:1, c * CM:(c + 1) * CM],
                                 ty2[0:1, b * T + s0 * W:b * T + s1 * W],
                                 start=False, stop=True)
                stg = stgp.tile([CM, cw], bf16, tag=f"stg{cw}")
                nc.scalar.activation(
                    stg[:], ps[:], AF.Sqrt,
                    bias=tx2[:, b * NC + c:b * NC + c + 1], scale=1.0)
                # write skewed: addr(b; i, s, f) =
                #   (s*nb+b)*TS*W + (L*s + c*CM + i)*W + f
                (dmaeng or nc.scalar).dma_start(
                    scr_ap(b * TS * W + c * CM * W
                           + s0 * (nb * TS + L) * W,
                           [[W, CM], [(nb * TS + L) * W, s1 - s0], [1, W]]),
                    stg[:])


            # ---- DP loop: one fused scan per step + boundary DMA per kb ----
            def dp_step(t):
                slot, pslot = t % nslot, (t - 1) % nslot
                data0 = ap_at(rows_h, pslot * C, [[pst, P], [2, W], [2, 2]])
                data1 = ap_at(ring_h, 0,
                              [[rpst, P], [1, W], [(1 + t % TR) * W, 2]])
                outap = ap_at(rows_h, slot * C + 1, [[pst, P], [1, 2 * W]])
                init = (zcol[:, 0:1] if t == 0 else rows[:, slot, 0:1])
                eng = nc.vector
                eng.add_instruction(
                    mybir.InstTensorScalarPtr(
                        name=nc.get_next_instruction_name(),
                        is_tensor_tensor_scan=True,
                        is_scalar_tensor_tensor=True,
                        op0=mn, op1=ad,
                        ins=[eng.lower_ap(data0),
                             eng.lower_ap_or_imm(init),
                             eng.lower_ap(data1)],
                        outs=[eng.lower_ap(outap)],
                    ))
                # boundary DMA for scans [u, u+kb): strip s-1 col-2W -> strip
                # s col0, +nb partitions, +L slots. Sources complete at this t.
                u = t + L - (kb - 1)
                if u >= L and u % kb == 0 and u < TS:
                    ssl = (u - L) % nslot
                    dsl = u % nslot
                    with nc.allow_non_contiguous_dma(
                            reason="tiny boundary column copy, kb elems/part"):
                        nc.sync.dma_start(
                            rows[nb:P, dsl:dsl + kb, 0:1],
                            rows[0:P - nb, ssl:ssl + kb, 2 * W:2 * W + 1])

            # sub-window reads (SW steps each), spread across the DP so the
            # DMA engines never burst long enough to delay a boundary packet
            SW = 16
            RA = TR - wt          # read-ahead in steps (ring wrap safe)
            n_sw = (TS + SW - 1) // SW
            prod_c = 1

            def sub_read(v):
                t0, t1 = v * SW, min((v + 1) * SW, TS)
                r0 = t0 % TR
                full = [s for s in range(S)
                        if L * s <= t0 and L * s + T >= t1]
                if full:
                    s_a, s_b = min(full), max(full)
                    nc.scalar.dma_start(
                        ring[s_a * nb:(s_b + 1) * nb, 1 + r0:1 + r0 + (t1 - t0), :],
                        scr_ap(s_a * nb * TS * W + t0 * W,
                               [[TS * W, (s_b - s_a + 1) * nb],
                                [1, (t1 - t0) * W]]))
                for s in range(S):
                    if s in full:
                        continue
                    v0, v1 = max(t0, L * s), min(t1, L * s + T)
                    if v0 >= v1:
                        continue
                    nc.scalar.dma_start(
                        ring[s * nb:(s + 1) * nb,
                             1 + r0 + (v0 - t0):1 + r0 + (v1 - t0), :],
                        scr_ap(s * nb * TS * W + v0 * W,
                               [[TS * W, nb], [1, (v1 - v0) * W]]))

            # chunk 0 full width gates the first scans; chunks 1-3 paced
            # inside the DP loop; sub-reads follow their producer writes
            for b in range(nb):
                produce_cols(0, b, 0, S, psd, dmaeng=nc.sync)

            def sw_ready(v):
                t1p = min((v + 1) * SW, TS)
                ch = min(NC - 1, (t1p - 1) // CM)
                return ch == 0 or prods_done >= ch * nb

            prods = [(c, b) for c in range(1, NC) for b in range(nb)]
            prods_done = 0
            sw_read = 0
            while sw_read * SW < 2 * wt and sw_ready(sw_read):
                sub_read(sw_read)
                sw_read += 1
            for t in range(TS):
                dp_step(t)
                if t % 5 == 2 and prods:
                    produce_cols(*prods.pop(0), 0, S, psd)
                    prods_done += 1
                if t % 2 == 1:
                    if (sw_read < n_sw and sw_read * SW < t + RA
                            and sw_ready(sw_read)):
                        sub_read(sw_read)
                        sw_read += 1

            # ---- extract answers: strip S-1, row T-1, col 2W ----
            nc.sync.dma_start(
                out[:], rows[(S - 1) * nb:P, (TS - 1) % nslot,
                             2 * W:2 * W + 1])

    nc.compile()
    return nc


_cache = {}

NB = B_FULL // NCORES
S_, W_, L_ = 8, 64, 8
NC_ = (T_FULL + 127) // 128
CM_ = T_FULL // NC_


def _get_nc():
    key = "full"
    if key not in _cache:
        _cache[key] = build_dtw(
            nb=NB, F=F_FULL, T=T_FULL, S=S_, W=W_, L=L_)
    return _cache[key]


def make_in_maps(x, y):
    """Host prep: shard over cores, transpose to [F, nb, T], cast bf16,
    precompute x2/y2 sums of squares in the kernel's layouts."""
    from ml_dtypes import bfloat16
    x = np.ascontiguousarray(x, dtype=np.float32)
    y = np.ascontiguousarray(y, dtype=np.float32)
    in_maps = []
    for c in range(NCORES):
        xs = x[c * NB:(c + 1) * NB]                      # [nb, F, T]
        ys = y[c * NB:(c + 1) * NB]
        xm2 = np.ascontiguousarray(
            (-2.0 * xs).transpose(1, 0, 2)).astype(bfloat16)
        yb = np.ascontiguousarray(ys.transpose(1, 0, 2)).astype(bfloat16)
        y2 = np.einsum("bft,bft->bt", ys, ys).reshape(1, -1).astype(bfloat16)
        x2b = np.einsum("bft,bft->bt", xs, xs)           # [nb, T]
        # x2 bias layout: [CM, nb*NC], column b*NC+c = x2[b, c*CM:(c+1)*CM]
        x2l = np.ascontiguousarray(
            x2b.reshape(NB, NC_, CM_).transpose(2, 0, 1)
        ).reshape(CM_, NB * NC_).astype(np.float32)
        in_maps.append({"xm2": xm2, "yb": yb, "y2": y2,
                        "x2": np.ascontiguousarray(x2l)})
    return in_maps


def kernel(x, y):
    from concourse.bass_utils import run_bass_kernel_spmd

    nc = _get_nc()
    in_maps = make_in_maps(x, y)
    res = run_bass_kernel_spmd(nc, in_maps, list(range(NCORES)))
    outs = [res.results[c]["out"].reshape(NB) for c in range(NCORES)]
    return np.concatenate(outs).astype(np.float32)
